# revision 29
# baseline (speedup 1.0000x reference)
"""Trainium2 Bass kernel for nn_DefAddkeysTransformer.

Sharding: one attention head per NeuronCore (8 heads / 8 cores).  Each core
gathers its head's deformable keys, computes the (reshape-scrambled) level
attention scores, the add_keys attention, a max-free softmax, and its head's
output contribution in transposed layout (C, Lq).  Host sums the 8 partial
outputs (the reference's per-head accumulation) and transposes back.

Perf notes vs the v1 kernel:
  - the 128 per-(lvl,t,p) indirect gathers are batched into 4 per-level
    indirect DMAs (the per-instruction SWDGE fixed cost dominated the old
    gather spine).
  - constants/weights arrive in a handful of packed blob DMAs instead of
    ~30 small ones (sync-engine descriptor dispatch was serializing).
  - the per-point value accumulation uses DVE scalar_tensor_tensor
    (V += wq_p * G_p) instead of diag-matmuls on the PE.
  - STB PSUM->SBUF copies are batched [128,512] instead of [128,64].
"""
import sys

sys.path.insert(0, '/opt/trn_rl_repo')

from contextlib import ExitStack

import numpy as np

import concourse.bass as bass
import concourse.tile as tile
from concourse import bacc
from concourse import mybir
from concourse.bass_utils import run_bass_kernel_spmd
from concourse.masks import make_identity

C = 256
H = 8
L = 4
P = 4
LQ = 1024
LX = 256
LEN_IN = 13294
NT = LQ // 128          # 8 query tiles
F32 = mybir.dt.float32
F32R = mybir.dt.float32r
F16 = mybir.dt.float16
I32 = mybir.dt.int32

DENSE_L3 = True   # build lvl-3 G via one-hot matmuls instead of gathers

# f32 blob A (idx-chain constants + W_off): column offsets
A_INVWH = 0
A_ISSF = 256
A_WH32 = 512
A_BOFF = 768
A_RP = 1024
A_WOFF0 = 1280
A_WOFF1 = 1312
A_IOTA3 = 1344
NA = 1346
# f32 blob B (late constants)
B_DMASK = 0
B_WMIX = 512          # 9 cols, chunk 0
B_WMIX1 = 521         # 9 cols, chunk 1
B_FLAG = 530
B_BV20 = 531
B_BV21 = 532
NB = 533
# f16 blob: qT16 | wvT16 | lvl-3 table slab (169 rows in 2 chunks)
S_QT16 = 0
S_WVT = 2048
S_K3 = 2560
NS = 3072
LV3_START = 13125
LV3_ROWS = 169
# f32r blob: simT(5 x 2ch x 256) | addkT | wv2T
R_SIM = 0
R_ADDK = 2560
R_WV2 = 3072
R_OH = 3584
NR = 3592


def build_kernel(nc: bass.Bass, tc: tile.TileContext, ctx: ExitStack):
    # ---------------- DRAM I/O ----------------
    d_qt32 = nc.dram_tensor("qt32", [128, 2 * LQ], F32, kind="ExternalInput").ap()
    d_cst32a = nc.dram_tensor("cst32a", [128, NA], F32, kind="ExternalInput").ap()
    d_cst32b = nc.dram_tensor("cst32b", [128, NB], F32, kind="ExternalInput").ap()
    d_csti = nc.dram_tensor("csti", [128, 256], I32, kind="ExternalInput").ap()
    d_cst16 = nc.dram_tensor("cst16", [128, NS], F16, kind="ExternalInput").ap()
    d_cstr = nc.dram_tensor("cstr", [128, NR], F32R, kind="ExternalInput").ap()
    d_bvd = nc.dram_tensor("bvd", [1, C], F32R, kind="ExternalInput").ap()
    d_flat = nc.dram_tensor("flatten16", [LEN_IN, C], F16, kind="ExternalInput").ap()
    d_out = nc.dram_tensor("outT", [C, LQ], F32, kind="ExternalOutput").ap()

    # ---------------- pools ----------------
    cst = ctx.enter_context(tc.tile_pool(name="cst", bufs=1))
    gpool = ctx.enter_context(tc.tile_pool(name="gpool", bufs=1))
    wrk = ctx.enter_context(tc.tile_pool(name="wrk", bufs=3))
    wrk8 = ctx.enter_context(tc.tile_pool(name="wrk8", bufs=8))
    wrk4 = ctx.enter_context(tc.tile_pool(name="wrk4", bufs=4))
    stsb = ctx.enter_context(tc.tile_pool(name="stsb", bufs=2))
    ps_st = ctx.enter_context(tc.tile_pool(name="ps_st", bufs=2, space="PSUM"))
    ps_c = ctx.enter_context(tc.tile_pool(name="ps_c", bufs=2, space="PSUM"))
    ps_tp = ctx.enter_context(tc.tile_pool(name="ps_tp", bufs=1, space="PSUM"))
    ps_o = ctx.enter_context(tc.tile_pool(name="ps_o", bufs=1, space="PSUM"))
    ps_g = ctx.enter_context(tc.tile_pool(name="ps_g", bufs=2, space="PSUM"))

    def csttile(shape, dtype=F32, tag=None):
        return cst.tile(shape, dtype, tag=tag, name=tag)

    # ------- phase 0: packed loads; offsets + gather indices ---------------
    CA = csttile([128, NA], tag="ca")
    nc.sync.dma_start(CA[:], d_cst32a[:])
    QTF = csttile([128, 2 * LQ], tag="qtf")     # [:, 1024*ch + q]
    nc.sync.dma_start(QTF[:], d_qt32[:])
    CI = csttile([128, 256], I32, tag="ci")
    nc.sync.dma_start(CI[:], d_csti[:])

    INVWH = CA[:, A_INVWH:A_INVWH + 256]
    ISSF = CA[:, A_ISSF:A_ISSF + 256]
    WH32 = CA[:, A_WH32:A_WH32 + 256]
    BOFF = CA[:, A_BOFF:A_BOFF + 256]
    RPB = CA[:, A_RP:A_RP + 256]
    WoffT = [CA[:, A_WOFF0:A_WOFF0 + 32], CA[:, A_WOFF1:A_WOFF1 + 32]]
    HL = CI[:, 0:128]
    LVST = CI[:, 128:256]

    OFFALL = csttile([128, 256], tag="offall")
    FLATB = csttile([128, 128], I32, tag="flatb")

    def off_tile(t):
        qsl = slice(128 * t, 128 * t + 128)
        pof = ps_c.tile([128, 32], F32, tag="pc", name="pc")
        for ch in range(2):
            nc.tensor.matmul(pof[:],
                             lhsT=QTF[:, 1024 * ch + 128 * t:1024 * ch + 128 * t + 128],
                             rhs=WoffT[ch], start=(ch == 0), stop=(ch == 1))
        nc.scalar.copy(OFFALL[:, 32 * t:32 * t + 32], pof[:])

    def idx_chain(hf):
        # faithful fp order: (q @ W.T + b), then x/wh (Newton-corrected
        # reciprocal multiply ~ IEEE division), then + rp; exact truncation.
        csl = slice(128 * hf, 128 * hf + 128)
        ksl = slice(64 * hf, 64 * hf + 64)
        t0 = wrk.tile([128, 128], F32, tag="ix0", name="ix0")
        nc.vector.tensor_tensor(out=t0[:], in0=OFFALL[:, csl], in1=BOFF[:, csl],
                                op=mybir.AluOpType.add)
        t1 = wrk.tile([128, 128], F32, tag="ix1", name="ix1")
        nc.vector.tensor_tensor(out=t1[:], in0=t0[:], in1=INVWH[:, csl],
                                op=mybir.AluOpType.mult)
        te = wrk.tile([128, 128], F32, tag="ixe", name="ixe")
        nc.vector.tensor_tensor(out=te[:], in0=t1[:], in1=WH32[:, csl],
                                op=mybir.AluOpType.mult)
        nc.vector.tensor_tensor(out=te[:], in0=t0[:], in1=te[:],
                                op=mybir.AluOpType.subtract)
        nc.vector.tensor_tensor(out=te[:], in0=te[:], in1=INVWH[:, csl],
                                op=mybir.AluOpType.mult)
        nc.vector.tensor_tensor(out=t1[:], in0=t1[:], in1=te[:],
                                op=mybir.AluOpType.add)
        nc.vector.tensor_tensor(out=t1[:], in0=t1[:], in1=RPB[:, csl],
                                op=mybir.AluOpType.add)
        nc.vector.tensor_scalar(out=t1[:], in0=t1[:], scalar1=0.999, scalar2=0.0,
                                op0=mybir.AluOpType.min, op1=mybir.AluOpType.max)
        nc.vector.tensor_tensor(out=t1[:], in0=t1[:], in1=ISSF[:, csl],
                                op=mybir.AluOpType.mult)
        ti = wrk.tile([128, 128], I32, tag="ix2", name="ix2")
        nc.vector.tensor_copy(ti[:], t1[:])      # f32 -> i32 (rounds on HW)
        fb = wrk.tile([128, 128], F32, tag="ixf", name="ixf")
        nc.vector.tensor_copy(fb[:], ti[:])
        gtf = wrk.tile([128, 128], F32, tag="ixg", name="ixg")
        nc.vector.tensor_tensor(out=gtf[:], in0=fb[:], in1=t1[:],
                                op=mybir.AluOpType.is_gt)
        gti = wrk.tile([128, 128], I32, tag="ixh", name="ixh")
        nc.vector.tensor_copy(gti[:], gtf[:])
        nc.vector.tensor_tensor(out=ti[:], in0=ti[:], in1=gti[:],
                                op=mybir.AluOpType.subtract)
        iv = ti[:].rearrange("p (k two) -> p k two", two=2)
        nc.vector.tensor_tensor(out=FLATB[:, ksl], in0=iv[:, :, 1],
                                in1=HL[:, ksl], op=mybir.AluOpType.mult)
        nc.vector.tensor_tensor(out=FLATB[:, ksl], in0=FLATB[:, ksl],
                                in1=iv[:, :, 0], op=mybir.AluOpType.add)
        nc.vector.tensor_tensor(out=FLATB[:, ksl], in0=FLATB[:, ksl],
                                in1=LVST[:, ksl], op=mybir.AluOpType.add)

    for t in range(4):
        off_tile(t)
    idx_chain(0)
    for t in range(4, NT):
        off_tile(t)
    idx_chain(1)
    lvls_sp = (1, 2) if DENSE_L3 else (1, 2, 3)
    GATHER_EARLY = [(0, t, p) for p in range(P) for t in range(4)]
    GATHER_REST = ([(0, t, p) for p in range(P) for t in range(4, NT)]
                   + [(lvl, t, p) for lvl in lvls_sp
                      for p in range(P) for t in range(NT)])

    # ------- phase 1: the gather spine -------------------------------------
    # HW indirect DMA honors exactly one offset per out partition, so each
    # instruction gathers 128 rows; 128 instructions total.  Ordered p-major
    # within each level so the level's score matmuls can start early; the
    # first 16 only need idx_chain(0).  GPSIMD is reserved for this spine
    # (no any-routed copies).
    # GL[lvl] column layout: 1024*t + 256*p + ch  (FLATB col order 16t+4l+p)
    GL = [gpool.tile([128, NT * 4 * C], F16, tag=f"g{lvl}", name=f"g{lvl}")
          for lvl in range(L)]

    def gather(lvl, t, p):
        col = 16 * t + 4 * lvl + p
        nc.gpsimd.indirect_dma_start(
            out=GL[lvl][:, 1024 * t + 256 * p:1024 * t + 256 * p + 256],
            out_offset=None,
            in_=d_flat[:],
            in_offset=bass.IndirectOffsetOnAxis(
                ap=FLATB[:, col:col + 1], axis=0),
        )

    def G(lvl, t, c0, c1):
        return GL[lvl][:, 1024 * t + c0:1024 * t + c1]

    for lvl, t, p in GATHER_EARLY + GATHER_REST:
        gather(lvl, t, p)

    # ------- phase 2: remaining packed loads + addk branch -----------------
    CS = csttile([128, NS], F16, tag="cs")
    nc.sync.dma_start(CS[:], d_cst16[:])
    CR = csttile([128, NR], F32R, tag="cr")
    nc.sync.dma_start(CR[:], d_cstr[:])
    CB = csttile([128, NB], tag="cb")
    nc.sync.dma_start(CB[:], d_cst32b[:])
    BVD = csttile([1, C], F32R, tag="bvd")
    nc.sync.dma_start(BVD[:], d_bvd[:])

    QT16 = [CS[:, 1024 * ch:1024 * ch + 1024] for ch in range(2)]
    QT16v = [QT16[ch].rearrange("p (a b) -> p a b", b=16) for ch in range(2)]
    WvT16 = [CS[:, S_WVT + 256 * ch:S_WVT + 256 * ch + 256] for ch in range(2)]
    SimT = [[CR[:, R_SIM + 512 * i + 256 * ch:R_SIM + 512 * i + 256 * ch + 256]
             for ch in range(2)] for i in range(5)]
    AddkT = [CR[:, R_ADDK + 256 * ch:R_ADDK + 256 * ch + 256] for ch in range(2)]
    Wv2T = [CR[:, R_WV2 + 256 * ch:R_WV2 + 256 * ch + 256] for ch in range(2)]
    DMASK = CB[:, B_DMASK:B_DMASK + 512]
    OH48 = CR[:, R_OH:R_OH + 8]
    WMIX = [CB[:, B_WMIX:B_WMIX + 9], CB[:, B_WMIX1:B_WMIX1 + 9]]
    FLG = CB[:, B_FLAG:B_FLAG + 1]
    BV2 = [CB[:, B_BV20:B_BV20 + 1], CB[:, B_BV21:B_BV21 + 1]]

    IDENT = csttile([128, 128], tag="ident")
    make_identity(nc, IDENT[:])
    IDENT16 = csttile([128, 128], F16, tag="ident16")
    nc.vector.tensor_copy(IDENT16[:], IDENT[:])
    NEG16 = csttile([128, 1], tag="neg16")
    nc.vector.memset(NEG16[:], -16.0)
    ONE1 = cst.tile([1, 128], F32, tag="one1", name="one1")
    nc.vector.memset(ONE1[:], 1.0)
    IOTA3 = CA[:, A_IOTA3:A_IOTA3 + 2]
    K3c = [CS[:, S_K3:S_K3 + 256], CS[0:41, S_K3 + 256:S_K3 + 512]]

    # lvl-3 flat indices as exact f32 (one strided convert, cols (t, p)),
    # then one transpose: rows32[b, s] = flat idx of block b (=4t+p), slot s
    FLAT3F = csttile([128, 32], tag="fl3f")
    nc.vector.tensor_scalar(
        out=FLAT3F[:].rearrange("p (t k) -> p t k", k=4),
        in0=FLATB[:].rearrange("p (t k) -> p t k", k=16)[:, :, 12:16],
        scalar1=-LV3_START, scalar2=None, op0=mybir.AluOpType.add)
    ROWS32 = csttile([32, 128], F16, tag="rows32")
    tpf = ps_tp.tile([128, 128], F32, tag="ptp", name="ptp")
    nc.tensor.transpose(out=tpf[:32, :], in_=FLAT3F[:], identity=IDENT[:])
    nc.vector.tensor_copy(ROWS32[:], tpf[:32, :])
    ONES32 = csttile([32, 128], F16, tag="ones32")
    nc.vector.memset(ONES32[:], 1.0)

    def dense_g3(t, p):
        # G3 block via one-hot decode against the 169-row lvl-3 slab:
        # OH[j, slot] = (flat_idx[slot] == LV3_START + j); G3 = OH^T-contract K3.
        b = 4 * t + p
        selb = wrk8.tile([32, 128], F16, tag="selb", name="selb")
        nc.scalar.activation(selb[:], ONES32[:], mybir.ActivationFunctionType.Copy,
                             scale=IDENT[:32, b:b + 1])
        psb = ps_g.tile([128, 128], F32, tag="pg", name="pg")
        nc.tensor.matmul(psb[:], lhsT=selb[:], rhs=ROWS32[:],
                         start=True, stop=True)
        oh0 = wrk.tile([128, 128], F16, tag="g3oh0", name="g3oh0")
        nc.vector.tensor_scalar(out=oh0[:], in0=psb[:], scalar1=IOTA3[:, 0:1],
                                scalar2=None, op0=mybir.AluOpType.is_equal)
        oh1 = wrk.tile([41, 128], F16, tag="g3oh1", name="g3oh1")
        nc.vector.tensor_scalar(out=oh1[:], in0=psb[:41, :], scalar1=IOTA3[:41, 1:2],
                                scalar2=None, op0=mybir.AluOpType.is_equal)
        g3p = ps_g.tile([128, 256], F32, tag="pg", name="pg")
        nc.tensor.matmul(g3p[:], lhsT=oh0[:], rhs=K3c[0], start=True, stop=False)
        nc.tensor.matmul(g3p[:], lhsT=oh1[:], rhs=K3c[1], start=False, stop=True)
        nc.scalar.copy(G(3, t, 256 * p, 256 * p + 256), g3p[:])

    # head_w softmax over the 9 mixture logits (per channel chunk)
    HWH = []
    BASE = []
    BV2HW = []
    for ch in range(2):
        mx = wrk.tile([128, 1], F32, tag="mx", name="mx")
        nc.vector.reduce_max(mx[:], WMIX[ch], axis=mybir.AxisListType.X)
        nmx = wrk.tile([128, 1], F32, tag="nmx", name="nmx")
        nc.vector.tensor_scalar_mul(nmx[:], mx[:], -1.0)
        ex = wrk.tile([128, 9], F32, tag="ex", name="ex")
        sm = wrk.tile([128, 1], F32, tag="sm", name="sm")
        nc.scalar.activation(ex[:], WMIX[ch], mybir.ActivationFunctionType.Exp,
                             bias=nmx[:], accum_out=sm[:])
        rs = wrk.tile([128, 1], F32, tag="rs", name="rs")
        nc.vector.reciprocal(rs[:], sm[:])
        hw = csttile([128, 2], tag=f"hw{ch}")
        nc.vector.tensor_scalar_mul(hw[:], ex[:, 0:2], rs[:])
        HWH.append(hw[:, 0:1])
        base = csttile([128, 1], tag=f"base{ch}")
        nc.vector.tensor_tensor(out=base[:], in0=hw[:, 1:2], in1=FLG,
                                op=mybir.AluOpType.mult)
        BASE.append(base)
        b2h = csttile([128, 1], tag=f"b2h{ch}")
        nc.vector.tensor_tensor(out=b2h[:], in0=BV2[ch], in1=hw[:, 0:1],
                                op=mybir.AluOpType.mult)
        BV2HW.append(b2h)

    # ki_T = simil_add applied to add_keys (c2 x Lx), fp16 for fast Tadd
    KiT = [csttile([128, LX], F16, tag=f"kit{m}") for m in range(2)]
    for m in range(2):
        pps = ps_c.tile([128, LX], F32, tag="pc", name="pc")
        for dch in range(2):
            nc.tensor.matmul(pps[:], lhsT=SimT[4][dch][:, 128 * m:128 * m + 128],
                             rhs=AddkT[dch], start=(dch == 0), stop=(dch == 1))
        nc.scalar.copy(KiT[m][:], pps[:])

    # v2 = add_keys @ W_val[2h+1].T   (Lx x C)
    V2 = [csttile([128, C], F32R, tag=f"v2{m}") for m in range(2)]
    for m in range(2):
        pps = ps_c.tile([128, C], F32, tag="pc", name="pc")
        for dch in range(2):
            nc.tensor.matmul(pps[:], lhsT=AddkT[dch][:, 128 * m:128 * m + 128],
                             rhs=Wv2T[dch], start=(dch == 0), stop=(dch == 1))
        nc.scalar.copy(V2[m][:], pps[:])

    # add_keys scores, exp(x-16), and early unnormalized transposes
    WADD = [csttile([128, LX], tag=f"wadd{t}") for t in range(NT)]
    ZADD = [csttile([128, 1], tag=f"zadd{t}") for t in range(NT)]
    ZL = [csttile([128, 1], tag=f"zl{t}") for t in range(NT)]
    V = [csttile([128, C], F16, tag=f"v{t}") for t in range(NT)]
    WAT = [cst.tile([128, LQ], F32R, tag=f"wat{ch}", name=f"wat{ch}")
           for ch in range(2)]
    for t in range(NT):
        qsl = slice(128 * t, 128 * t + 128)
        pta = ps_c.tile([128, LX], F32, tag="pc", name="pc")
        for ch in range(2):
            nc.tensor.matmul(pta[:], lhsT=QT16[ch][:, qsl], rhs=KiT[ch][:],
                             start=(ch == 0), stop=(ch == 1))
        nc.scalar.activation(WADD[t][:], pta[:], mybir.ActivationFunctionType.Exp,
                             bias=NEG16[:], accum_out=ZADD[t][:])
        for ch in range(2):
            fsl = slice(128 * ch, 128 * ch + 128)
            tp3 = ps_tp.tile([128, 128], F32, tag="ptp", name="ptp")
            nc.tensor.transpose(out=tp3[:], in_=WADD[t][:, fsl], identity=IDENT[:])
            nc.scalar.copy(WAT[ch][:, qsl], tp3[:])

    # ------- phase 3: per-level scores + DVE value accumulation ------------
    # Level order: 3 first (its G comes from the dense one-hot path and is
    # ready before any gathers land), then the gathered levels; the last
    # level also performs the per-tile normalization/transpose tail.
    S1T = cst.tile([1, LQ], F32R, tag="s1t", name="s1t")
    ZR = cst.tile([1, LQ], F32, tag="zr", name="zr")
    VT = [cst.tile([128, LQ], F16, tag=f"vt{ch}", name=f"vt{ch}")
          for ch in range(2)]
    SALL2 = [cst.tile([4, LQ], F32, tag=f"sall{k}", name=f"sall{k}")
             for k in range(2)]
    LVL_ORDER = (3, 0, 1, 2) if DENSE_L3 else (0, 1, 2, 3)
    for li, lvl in enumerate(LVL_ORDER):
        first, lastl = (li == 0), (li == L - 1)
        SALL = SALL2[li % 2]
        if lvl == 3 and DENSE_L3:
            for t in range(NT):
                for p in range(P):
                    dense_g3(t, p)
        STB = [[stsb.tile([128, 512], F32R, tag=f"stb{b8}_{dch}",
                          name=f"stb{b8}_{dch}") for dch in range(2)]
               for b8 in range(2)]
        for b8 in range(2):
            for dch in range(2):
                sps = ps_st.tile([128, 512], F32, tag="pst", name="pst")
                for qb in range(8):
                    ql = 8 * b8 + qb
                    b, pp = ql % 4, ql // 4
                    for c2 in range(2):
                        nc.tensor.matmul(
                            sps[:, 64 * qb:64 * qb + 64],
                            lhsT=G(lvl, 2 * b + c2,
                                   256 * pp + 128 * dch, 256 * pp + 128 * dch + 128),
                            rhs=QT16v[c2][:, :, ql],
                            start=(c2 == 0), stop=(c2 == 1))
                nc.scalar.copy(STB[b8][dch][:], sps[:])
        for b8 in range(2):
            scp = ps_o.tile([4, 512], F32, tag="po", name="po")
            for ich in range(2):
                cps = ps_c.tile([128, 512], F32, tag="pc", name="pc")
                isl = slice(128 * ich, 128 * ich + 128)
                for dch in range(2):
                    nc.tensor.matmul(cps[:], lhsT=SimT[lvl][dch][:, isl],
                                     rhs=STB[b8][dch][:],
                                     start=(dch == 0), stop=(dch == 1))
                mskb = wrk4.tile([128, 512], F32R, tag="mskb", name="mskb")
                nc.vector.tensor_tensor(out=mskb[:], in0=cps[:], in1=DMASK,
                                        op=mybir.AluOpType.mult)
                nc.tensor.matmul(scp[:], lhsT=OH48[:, 4 * ich:4 * ich + 4],
                                 rhs=mskb[:], start=(ich == 0), stop=(ich == 1))
            sview = SALL[:].rearrange("p (t s) -> p s t", s=16)
            nc.scalar.copy(sview[:, 8 * b8:8 * b8 + 8, :], scp[:])

        for t in range(NT):
            tps = ps_tp.tile([128, 128], F32, tag="ptp", name="ptp")
            nc.tensor.transpose(out=tps[:, :4], in_=SALL[:, 128 * t:128 * t + 128],
                                identity=IDENT[:4, :4])
            wq = wrk8.tile([128, 4], F32, tag="wq", name="wq")
            zp = wrk8.tile([128, 1], F32, tag="zp", name="zp")
            nc.scalar.activation(wq[:], tps[:, :4], mybir.ActivationFunctionType.Exp,
                                 bias=NEG16[:], accum_out=zp[:])
            if first:
                nc.vector.tensor_copy(ZL[t][:], zp[:])
                nc.vector.tensor_scalar_mul(V[t][:], G(lvl, t, 0, 256), wq[:, 0:1])
            else:
                nc.vector.tensor_tensor(out=ZL[t][:], in0=ZL[t][:], in1=zp[:],
                                        op=mybir.AluOpType.add)
                nc.vector.scalar_tensor_tensor(
                    out=V[t][:], in0=G(lvl, t, 0, 256), scalar=wq[:, 0:1],
                    in1=V[t][:], op0=mybir.AluOpType.mult, op1=mybir.AluOpType.add)
            for p in range(1, P):
                nc.vector.scalar_tensor_tensor(
                    out=V[t][:], in0=G(lvl, t, 256 * p, 256 * p + 256),
                    scalar=wq[:, p:p + 1], in1=V[t][:],
                    op0=mybir.AluOpType.mult, op1=mybir.AluOpType.add)
            if lastl:
                # per-tile normalization/transpose tail, overlapped with the
                # remaining tiles' score/value work
                qsl = slice(128 * t, 128 * t + 128)
                zt = wrk.tile([128, 1], F32, tag="zt", name="zt")
                nc.vector.tensor_tensor(out=zt[:], in0=ZL[t][:], in1=ZADD[t][:],
                                        op=mybir.AluOpType.add)
                rz = wrk.tile([128, 1], F32, tag="rz", name="rz")
                nc.vector.reciprocal(rz[:], zt[:])
                tpsz = ps_tp.tile([128, 128], F32, tag="ptp", name="ptp")
                nc.tensor.transpose(out=tpsz[:1, :], in_=ZL[t][:], identity=IDENT[:])
                nc.scalar.copy(S1T[:, qsl], tpsz[:1, :])
                tpz = ps_tp.tile([128, 128], F32, tag="ptp", name="ptp")
                nc.tensor.transpose(out=tpz[:1, :], in_=rz[:], identity=IDENT[:])
                nc.scalar.copy(ZR[:, qsl], tpz[:1, :])
                for ch in range(2):
                    fsl = slice(128 * ch, 128 * ch + 128)
                    tp2 = ps_g.tile([128, 128], F16, tag="pg", name="pg")
                    nc.tensor.transpose(out=tp2[:], in_=V[t][:, fsl],
                                        identity=IDENT16[:])
                    nc.scalar.copy(VT[ch][:, qsl], tp2[:])

    # ------- phase 4 (tail): output matmuls ---------------------------------
    RES = [cst.tile([128, LQ], F32, tag=f"res{m}", name=f"res{m}") for m in range(2)]
    for n in range(2):
        nsl = slice(512 * n, 512 * n + 512)
        for m in range(2):
            msl = slice(128 * m, 128 * m + 128)
            rzb = ps_st.tile([128, 512], F32, tag="pst", name="pst")
            nc.tensor.matmul(rzb[:], lhsT=ONE1[:], rhs=ZR[:, nsl],
                             start=True, stop=True)
            ops = ps_o.tile([128, 512], F32, tag="po", name="po")
            nc.tensor.matmul(ops[:], lhsT=WvT16[0][:, msl], rhs=VT[0][:, nsl],
                             start=True, stop=False)
            nc.tensor.matmul(ops[:], lhsT=WvT16[1][:, msl], rhs=VT[1][:, nsl],
                             start=False, stop=False)
            nc.tensor.matmul(ops[:], lhsT=BVD[:, msl], rhs=S1T[:, nsl],
                             start=False, stop=False)
            nc.tensor.matmul(ops[:], lhsT=V2[0][:, msl], rhs=WAT[0][:, nsl],
                             start=False, stop=False)
            nc.tensor.matmul(ops[:], lhsT=V2[1][:, msl], rhs=WAT[1][:, nsl],
                             start=False, stop=True)
            sc1 = wrk.tile([128, 512], F32, tag="sc1", name="sc1")
            nc.scalar.activation(sc1[:], ops[:],
                                 mybir.ActivationFunctionType.Copy, scale=HWH[m])
            nc.vector.tensor_tensor(out=sc1[:], in0=sc1[:], in1=rzb[:],
                                    op=mybir.AluOpType.mult)
            bt = wrk.tile([128, 512], F32, tag="bt", name="bt")
            nc.scalar.activation(bt[:], QTF[:, 1024 * m + 512 * n:1024 * m + 512 * n + 512],
                                 mybir.ActivationFunctionType.Copy, scale=BASE[m][:])
            nc.vector.tensor_tensor(out=sc1[:], in0=sc1[:], in1=bt[:],
                                    op=mybir.AluOpType.add)
            nc.vector.tensor_scalar_add(RES[m][:, nsl], sc1[:], BV2HW[m][:])
            nc.sync.dma_start(d_out[msl, nsl], RES[m][:, nsl])


def _host_prepare(inputs):
    """Build per-core input maps from the full problem inputs."""
    q = np.asarray(inputs["query"], np.float32)[0]            # (1024, 256)
    rp = np.asarray(inputs["reference_points"], np.float32)[0]
    flat = np.ascontiguousarray(np.asarray(inputs["input_flatten"], np.float32)[0])
    iss = np.asarray(inputs["input_spatial_shapes"], np.int32)
    addk = np.asarray(inputs["add_keys"], np.float32)[0]
    lvst = np.asarray(inputs["input_level_start_index"], np.int32)
    W_off = np.asarray(inputs["W_off"], np.float32)
    b_off = np.asarray(inputs["b_off"], np.float32)
    W_attn = np.asarray(inputs["W_attn"], np.float32)
    W_val = np.asarray(inputs["W_val"], np.float32)
    b_val = np.asarray(inputs["b_val"], np.float32)
    W_mix = np.asarray(inputs["W_mix"], np.float32)

    iss_f = iss.astype(np.float32)
    wh = iss_f[:, ::-1]                                       # (W_l, H_l)
    inv_wh32 = np.repeat((1.0 / wh)[:, None, :], P, 1).reshape(32)
    iss32 = np.repeat(iss_f[:, None, :], P, 1).reshape(32)
    hl16 = np.repeat(iss[:, 0][:, None], P, 1).reshape(16)
    lv16 = np.repeat(lvst[:, None], P, 1).reshape(16)
    rp_rep = np.repeat(rp[:, :, None, :], P, 2).reshape(LQ, 32)
    wh32 = np.repeat(wh[:, None, :], P, 1).reshape(32)

    qT = np.ascontiguousarray(q.T)                            # (256, 1024)
    qt32 = np.concatenate([qT[:128], qT[128:]], axis=1)       # (128, 2048)
    qt16 = qt32.astype(np.float16)

    # ---- blob A (f32): idx-chain constants + W_off ----
    ca = np.zeros((128, NA), np.float32)
    ca[:, A_INVWH:A_INVWH + 256] = np.tile(inv_wh32, (128, 8))
    ca[:, A_ISSF:A_ISSF + 256] = np.tile(iss32, (128, 8))
    ca[:, A_WH32:A_WH32 + 256] = np.tile(wh32, (128, 8))
    ca[:, A_RP:A_RP + 256] = rp_rep.reshape(8, 128, 32).transpose(1, 0, 2).reshape(128, 256)
    iota3 = np.zeros((128, 2), np.float32)
    iota3[:, 0] = np.arange(128)
    iota3[:, 1] = 128 + np.arange(128)
    ca[:, A_IOTA3:A_IOTA3 + 2] = iota3

    # ---- blob B (f32): dmask/oh48/wmix/flag/bv2 ----
    cb = np.zeros((128, NB), np.float32)
    dm = np.zeros((128, 512), np.float32)
    for rr in range(128):
        dm[rr, rr % 64::64] = 1.0
    cb[:, B_DMASK:B_DMASK + 512] = dm
    oh = np.zeros((128, 8), np.float32)
    for rr in range(128):
        oh[rr, rr // 64] = 1.0          # ich 0: i//64 = p
        oh[rr, 4 + 2 + rr // 64] = 1.0  # ich 1: p = 2 + i'//64

    # ---- blob I (i32) ----
    ci = np.zeros((128, 256), np.int32)
    ci[:, 0:128] = np.tile(hl16, (128, 8))
    ci[:, 128:256] = np.tile(lv16, (128, 8))

    addkT = np.ascontiguousarray(addk.T)                      # (256, 256)

    in_maps = []
    for h in range(H):
        boff = b_off[32 * h:32 * h + 32]
        order = [h, 8] + [k for k in range(9) if k not in (h, 8)]
        wmix_r = np.ascontiguousarray(W_mix[:, order])        # (256, 9)

        cah = ca.copy()
        cah[:, A_BOFF:A_BOFF + 256] = np.tile(boff, (128, 8))
        woffT = np.ascontiguousarray(W_off[32 * h:32 * h + 32].T)  # (256, 32)
        cah[:, A_WOFF0:A_WOFF0 + 32] = woffT[:128]
        cah[:, A_WOFF1:A_WOFF1 + 32] = woffT[128:]

        cbh = cb.copy()
        cbh[:, B_WMIX:B_WMIX + 9] = wmix_r[:128]
        cbh[:, B_WMIX1:B_WMIX1 + 9] = wmix_r[128:]
        cbh[:, B_FLAG] = 1.0 if h == 0 else 0.0
        bv2 = b_val[2 * h + 1]                                # (256,)
        cbh[:, B_BV20] = bv2[:128]
        cbh[:, B_BV21] = bv2[128:]

        wvT = np.ascontiguousarray(W_val[2 * h].T).astype(np.float16)  # (256,256)
        cs = np.zeros((128, NS), np.float16)
        cs[:, S_QT16:S_QT16 + 2048] = qt16
        cs[:, S_WVT:S_WVT + 256] = wvT[:128]
        cs[:, S_WVT + 256:S_WVT + 512] = wvT[128:]
        k3 = flat.astype(np.float16)[LV3_START:LV3_START + LV3_ROWS]
        cs[:, S_K3:S_K3 + 256] = k3[:128]
        cs[:41, S_K3 + 256:S_K3 + 512] = k3[128:]

        wattnT = np.transpose(W_attn[4 * h:4 * h + 5], (0, 2, 1))  # (5,256,256)
        cr = np.zeros((128, NR), np.float32)
        for i in range(5):
            cr[:, R_SIM + 512 * i:R_SIM + 512 * i + 256] = wattnT[i, :128]
            cr[:, R_SIM + 512 * i + 256:R_SIM + 512 * i + 512] = wattnT[i, 128:]
        cr[:, R_ADDK:R_ADDK + 256] = addkT[:128]
        cr[:, R_ADDK + 256:R_ADDK + 512] = addkT[128:]
        wv2T = np.ascontiguousarray(W_val[2 * h + 1].T)
        cr[:, R_WV2:R_WV2 + 256] = wv2T[:128]
        cr[:, R_WV2 + 256:R_WV2 + 512] = wv2T[128:]
        cr[:, R_OH:R_OH + 8] = oh

        m = {
            "qt32": qt32,
            "cst32a": cah,
            "cst32b": cbh,
            "csti": ci,
            "cst16": cs,
            "cstr": cr,
            "bvd": (b_val[2 * h] - b_val[2 * h + 1]).reshape(1, C).astype(np.float32),
            "flatten16": flat.astype(np.float16),
        }
        in_maps.append(m)
    return in_maps


_CACHE = {}


def _get_nc():
    if "nc" not in _CACHE:
        nc = bacc.Bacc("TRN2", target_bir_lowering=False, debug=False)
        with tile.TileContext(nc) as tc:
            with ExitStack() as ctx:
                build_kernel(nc, tc, ctx)
        nc.compile()
        _CACHE["nc"] = nc
    return _CACHE["nc"]


def kernel(**inputs):
    nc = _get_nc()
    in_maps = _host_prepare(inputs)
    res = run_bass_kernel_spmd(nc, in_maps, core_ids=list(range(8)))
    total = np.zeros((C, LQ), np.float32)
    for h in range(H):
        total = total + res.results[h]["outT"]
    return np.ascontiguousarray(total.T)[None].astype(np.float32)


if __name__ == "__main__":
    import reference as R
    import jax.numpy as jnp

    inputs = {k: np.asarray(v) for k, v in R.setup_inputs().items()}
    out = kernel(**inputs)
    exp = np.asarray(R.reference(**{k: jnp.asarray(v) for k, v in inputs.items()}))
    err = np.abs(out - exp)
    scale = np.abs(exp).max()
    print("max abs err:", err.max(), "scale:", scale, "rel:", err.max() / scale)


# revision 32
# speedup vs baseline: 1.1968x; 1.1968x over previous
"""Trainium2 Bass kernel for nn_DefAddkeysTransformer.

Sharding: one attention head per NeuronCore (8 heads / 8 cores).  Each core
gathers its head's deformable keys, computes the (reshape-scrambled) level
attention scores, the add_keys attention, a max-free softmax, and its head's
output contribution in transposed layout (C, Lq).  Host sums the 8 partial
outputs (the reference's per-head accumulation) and transposes back.

Perf notes vs the v1 kernel:
  - the 128 per-(lvl,t,p) indirect gathers are batched into 4 per-level
    indirect DMAs (the per-instruction SWDGE fixed cost dominated the old
    gather spine).
  - constants/weights arrive in a handful of packed blob DMAs instead of
    ~30 small ones (sync-engine descriptor dispatch was serializing).
  - the per-point value accumulation uses DVE scalar_tensor_tensor
    (V += wq_p * G_p) instead of diag-matmuls on the PE.
  - STB PSUM->SBUF copies are batched [128,512] instead of [128,64].
"""
import sys

sys.path.insert(0, '/opt/trn_rl_repo')

from contextlib import ExitStack

import numpy as np

import concourse.bass as bass
import concourse.tile as tile
from concourse import bacc
from concourse import mybir
from concourse.bass_utils import run_bass_kernel_spmd
from concourse.masks import make_identity

C = 256
H = 8
L = 4
P = 4
LQ = 1024
LX = 256
LEN_IN = 13294
NT = LQ // 128          # 8 query tiles
F32 = mybir.dt.float32
F32R = mybir.dt.float32r
F16 = mybir.dt.float16
I32 = mybir.dt.int32

DENSE_L3 = True   # build lvl-3 G via one-hot matmuls instead of gathers

# f32 blob A (idx-chain constants + W_off): column offsets
A_INVWH = 0
A_ISSF = 256
A_WH32 = 512
A_BOFF = 768
A_RP = 1024
A_WOFF0 = 1280
A_WOFF1 = 1312
A_IOTA3 = 1344
NA = 1346
# f32 blob B (late constants)
B_DMASK = 0
B_WMIX = 512          # 9 cols, chunk 0
B_WMIX1 = 521         # 9 cols, chunk 1
B_FLAG = 530
B_BV20 = 531
B_BV21 = 532
NB = 533
# f16 blob: qT16 | wvT16 | lvl-3 table slab (169 rows in 2 chunks)
S_QT16 = 0
S_WVT = 2048
S_K3 = 2560
NS = 3072
LV3_START = 13125
LV3_ROWS = 169
# f32r blob: simT(5 x 2ch x 256) | addkT | wv2T
R_SIM = 0
R_ADDK = 2560
R_WV2 = 3072
R_OH = 3584
NR = 3592


def build_kernel(nc: bass.Bass, tc: tile.TileContext, ctx: ExitStack):
    # ---------------- DRAM I/O ----------------
    d_qt32 = nc.dram_tensor("qt32", [128, 2 * LQ], F32, kind="ExternalInput").ap()
    d_cst32a = nc.dram_tensor("cst32a", [128, NA], F32, kind="ExternalInput").ap()
    d_cst32b = nc.dram_tensor("cst32b", [128, NB], F32, kind="ExternalInput").ap()
    d_csti = nc.dram_tensor("csti", [128, 256], I32, kind="ExternalInput").ap()
    d_cst16 = nc.dram_tensor("cst16", [128, NS], F16, kind="ExternalInput").ap()
    d_cstr = nc.dram_tensor("cstr", [128, NR], F32R, kind="ExternalInput").ap()
    d_bvd = nc.dram_tensor("bvd", [1, C], F32R, kind="ExternalInput").ap()
    d_flat = nc.dram_tensor("flatten16", [LEN_IN, C], F16, kind="ExternalInput").ap()
    d_out = nc.dram_tensor("outT", [C, LQ], F16, kind="ExternalOutput").ap()

    # ---------------- pools ----------------
    cst = ctx.enter_context(tc.tile_pool(name="cst", bufs=1))
    gpool = ctx.enter_context(tc.tile_pool(name="gpool", bufs=1))
    wrk = ctx.enter_context(tc.tile_pool(name="wrk", bufs=3))
    stsb = ctx.enter_context(tc.tile_pool(name="stsb", bufs=3))
    ps_st = ctx.enter_context(tc.tile_pool(name="ps_st", bufs=2, space="PSUM"))
    ps_c = ctx.enter_context(tc.tile_pool(name="ps_c", bufs=2, space="PSUM"))
    ps_tp = ctx.enter_context(tc.tile_pool(name="ps_tp", bufs=1, space="PSUM"))
    ps_o = ctx.enter_context(tc.tile_pool(name="ps_o", bufs=1, space="PSUM"))
    ps_g = ctx.enter_context(tc.tile_pool(name="ps_g", bufs=2, space="PSUM"))

    def csttile(shape, dtype=F32, tag=None):
        return cst.tile(shape, dtype, tag=tag, name=tag)

    # ------- phase 0: packed loads; offsets + gather indices ---------------
    CA = csttile([128, NA], tag="ca")
    nc.sync.dma_start(CA[:], d_cst32a[:])
    QTF = csttile([128, 2 * LQ], tag="qtf")     # [:, 1024*ch + q]
    nc.sync.dma_start(QTF[:], d_qt32[:])
    CI = csttile([128, 256], I32, tag="ci")
    nc.sync.dma_start(CI[:], d_csti[:])

    INVWH = CA[:, A_INVWH:A_INVWH + 256]
    ISSF = CA[:, A_ISSF:A_ISSF + 256]
    WH32 = CA[:, A_WH32:A_WH32 + 256]
    BOFF = CA[:, A_BOFF:A_BOFF + 256]
    RPB = CA[:, A_RP:A_RP + 256]
    WoffT = [CA[:, A_WOFF0:A_WOFF0 + 32], CA[:, A_WOFF1:A_WOFF1 + 32]]
    HL = CI[:, 0:128]
    LVST = CI[:, 128:256]

    OFFALL = csttile([128, 256], tag="offall")
    FLATB = csttile([128, 128], I32, tag="flatb")

    def off_tile(t):
        qsl = slice(128 * t, 128 * t + 128)
        pof = ps_c.tile([128, 32], F32, tag="pc", name="pc")
        for ch in range(2):
            nc.tensor.matmul(pof[:],
                             lhsT=QTF[:, 1024 * ch + 128 * t:1024 * ch + 128 * t + 128],
                             rhs=WoffT[ch], start=(ch == 0), stop=(ch == 1))
        nc.scalar.copy(OFFALL[:, 32 * t:32 * t + 32], pof[:])

    def idx_chain(hf):
        # faithful fp order: (q @ W.T + b), then x/wh (Newton-corrected
        # reciprocal multiply ~ IEEE division), then + rp; exact truncation.
        csl = slice(128 * hf, 128 * hf + 128)
        ksl = slice(64 * hf, 64 * hf + 64)
        t0 = wrk.tile([128, 128], F32, tag="ix0", name="ix0")
        nc.vector.tensor_tensor(out=t0[:], in0=OFFALL[:, csl], in1=BOFF[:, csl],
                                op=mybir.AluOpType.add)
        t1 = wrk.tile([128, 128], F32, tag="ix1", name="ix1")
        nc.vector.tensor_tensor(out=t1[:], in0=t0[:], in1=INVWH[:, csl],
                                op=mybir.AluOpType.mult)
        te = wrk.tile([128, 128], F32, tag="ixe", name="ixe")
        nc.vector.tensor_tensor(out=te[:], in0=t1[:], in1=WH32[:, csl],
                                op=mybir.AluOpType.mult)
        nc.vector.tensor_tensor(out=te[:], in0=t0[:], in1=te[:],
                                op=mybir.AluOpType.subtract)
        nc.vector.tensor_tensor(out=te[:], in0=te[:], in1=INVWH[:, csl],
                                op=mybir.AluOpType.mult)
        nc.vector.tensor_tensor(out=t1[:], in0=t1[:], in1=te[:],
                                op=mybir.AluOpType.add)
        nc.vector.tensor_tensor(out=t1[:], in0=t1[:], in1=RPB[:, csl],
                                op=mybir.AluOpType.add)
        nc.vector.tensor_scalar(out=t1[:], in0=t1[:], scalar1=0.999, scalar2=0.0,
                                op0=mybir.AluOpType.min, op1=mybir.AluOpType.max)
        nc.vector.tensor_tensor(out=t1[:], in0=t1[:], in1=ISSF[:, csl],
                                op=mybir.AluOpType.mult)
        ti = wrk.tile([128, 128], I32, tag="ix2", name="ix2")
        nc.vector.tensor_copy(ti[:], t1[:])      # f32 -> i32 (rounds on HW)
        fb = wrk.tile([128, 128], F32, tag="ixf", name="ixf")
        nc.vector.tensor_copy(fb[:], ti[:])
        gtf = wrk.tile([128, 128], F32, tag="ixg", name="ixg")
        nc.vector.tensor_tensor(out=gtf[:], in0=fb[:], in1=t1[:],
                                op=mybir.AluOpType.is_gt)
        gti = wrk.tile([128, 128], I32, tag="ixh", name="ixh")
        nc.vector.tensor_copy(gti[:], gtf[:])
        nc.vector.tensor_tensor(out=ti[:], in0=ti[:], in1=gti[:],
                                op=mybir.AluOpType.subtract)
        iv = ti[:].rearrange("p (k two) -> p k two", two=2)
        nc.vector.tensor_tensor(out=FLATB[:, ksl], in0=iv[:, :, 1],
                                in1=HL[:, ksl], op=mybir.AluOpType.mult)
        nc.vector.tensor_tensor(out=FLATB[:, ksl], in0=FLATB[:, ksl],
                                in1=iv[:, :, 0], op=mybir.AluOpType.add)
        nc.vector.tensor_tensor(out=FLATB[:, ksl], in0=FLATB[:, ksl],
                                in1=LVST[:, ksl], op=mybir.AluOpType.add)

    for t in range(4):
        off_tile(t)
    idx_chain(0)
    for t in range(4, NT):
        off_tile(t)
    idx_chain(1)
    lvls_sp = (1, 2) if DENSE_L3 else (1, 2, 3)
    GATHER_EARLY = [(0, t, p) for p in range(P) for t in range(4)]
    GATHER_REST = ([(0, t, p) for p in range(P) for t in range(4, NT)]
                   + [(lvl, t, p) for lvl in lvls_sp
                      for p in range(P) for t in range(NT)])

    # ------- phase 1: the gather spine -------------------------------------
    # HW indirect DMA honors exactly one offset per out partition, so each
    # instruction gathers 128 rows; 128 instructions total.  Ordered p-major
    # within each level so the level's score matmuls can start early; the
    # first 16 only need idx_chain(0).  GPSIMD is reserved for this spine
    # (no any-routed copies).
    # GL[lvl] column layout: 1024*t + 256*p + ch  (FLATB col order 16t+4l+p)
    GL = [gpool.tile([128, NT * 4 * C], F16, tag=f"g{lvl}", name=f"g{lvl}")
          for lvl in range(L)]

    def gather(lvl, t, p):
        col = 16 * t + 4 * lvl + p
        nc.gpsimd.indirect_dma_start(
            out=GL[lvl][:, 1024 * t + 256 * p:1024 * t + 256 * p + 256],
            out_offset=None,
            in_=d_flat[:],
            in_offset=bass.IndirectOffsetOnAxis(
                ap=FLATB[:, col:col + 1], axis=0),
        )

    def G(lvl, t, c0, c1):
        return GL[lvl][:, 1024 * t + c0:1024 * t + c1]

    for lvl, t, p in GATHER_EARLY + GATHER_REST:
        gather(lvl, t, p)

    # ------- phase 2: remaining packed loads + addk branch -----------------
    CS = csttile([128, NS], F16, tag="cs")
    nc.sync.dma_start(CS[:], d_cst16[:])
    CR = csttile([128, NR], F32R, tag="cr")
    nc.sync.dma_start(CR[:], d_cstr[:])
    CB = csttile([128, NB], tag="cb")
    nc.sync.dma_start(CB[:], d_cst32b[:])
    BVD = csttile([1, C], F32R, tag="bvd")
    nc.sync.dma_start(BVD[:], d_bvd[:])

    QT16 = [CS[:, 1024 * ch:1024 * ch + 1024] for ch in range(2)]
    QT16v = [QT16[ch].rearrange("p (a b) -> p a b", b=16) for ch in range(2)]
    WvT16 = [CS[:, S_WVT + 256 * ch:S_WVT + 256 * ch + 256] for ch in range(2)]
    SimT = [[CR[:, R_SIM + 512 * i + 256 * ch:R_SIM + 512 * i + 256 * ch + 256]
             for ch in range(2)] for i in range(5)]
    AddkT = [CR[:, R_ADDK + 256 * ch:R_ADDK + 256 * ch + 256] for ch in range(2)]
    Wv2T = [CR[:, R_WV2 + 256 * ch:R_WV2 + 256 * ch + 256] for ch in range(2)]
    DMASK = CB[:, B_DMASK:B_DMASK + 512]
    OH48 = CR[:, R_OH:R_OH + 8]
    WMIX = [CB[:, B_WMIX:B_WMIX + 9], CB[:, B_WMIX1:B_WMIX1 + 9]]
    FLG = CB[:, B_FLAG:B_FLAG + 1]
    BV2 = [CB[:, B_BV20:B_BV20 + 1], CB[:, B_BV21:B_BV21 + 1]]

    IDENT = csttile([128, 128], tag="ident")
    make_identity(nc, IDENT[:])
    IDENT16 = csttile([128, 128], F16, tag="ident16")
    nc.vector.tensor_copy(IDENT16[:], IDENT[:])
    NEG16 = csttile([128, 1], tag="neg16")
    nc.vector.memset(NEG16[:], -16.0)
    ONE1 = cst.tile([1, 128], F32, tag="one1", name="one1")
    nc.vector.memset(ONE1[:], 1.0)
    IOTA3 = CA[:, A_IOTA3:A_IOTA3 + 2]
    K3c = [CS[:, S_K3:S_K3 + 256], CS[0:41, S_K3 + 256:S_K3 + 512]]

    # lvl-3 flat indices as exact f32 (one strided convert, cols (t, p)),
    # then one transpose: rows32[b, s] = flat idx of block b (=4t+p), slot s
    FLAT3F = csttile([128, 32], tag="fl3f")
    nc.vector.tensor_scalar(
        out=FLAT3F[:].rearrange("p (t k) -> p t k", k=4),
        in0=FLATB[:].rearrange("p (t k) -> p t k", k=16)[:, :, 12:16],
        scalar1=-LV3_START, scalar2=None, op0=mybir.AluOpType.add)
    ROWS32 = csttile([32, 128], F16, tag="rows32")
    tpf = ps_tp.tile([128, 128], F32, tag="ptp", name="ptp")
    nc.tensor.transpose(out=tpf[:32, :], in_=FLAT3F[:], identity=IDENT[:])
    nc.vector.tensor_copy(ROWS32[:], tpf[:32, :])
    ONES32 = csttile([32, 128], F16, tag="ones32")
    nc.vector.memset(ONES32[:], 1.0)

    def dense_g3(t, p):
        # G3 block via one-hot decode against the 169-row lvl-3 slab:
        # OH[j, slot] = (flat_idx[slot] == LV3_START + j); G3 = OH^T-contract K3.
        b = 4 * t + p
        selb = wrk.tile([32, 128], F16, tag="selb", name="selb")
        nc.scalar.activation(selb[:], ONES32[:], mybir.ActivationFunctionType.Copy,
                             scale=IDENT[:32, b:b + 1])
        psb = ps_g.tile([128, 128], F32, tag="pg", name="pg")
        nc.tensor.matmul(psb[:], lhsT=selb[:], rhs=ROWS32[:],
                         start=True, stop=True)
        oh0 = wrk.tile([128, 128], F16, tag="g3oh0", name="g3oh0")
        nc.vector.tensor_scalar(out=oh0[:], in0=psb[:], scalar1=IOTA3[:, 0:1],
                                scalar2=None, op0=mybir.AluOpType.is_equal)
        oh1 = wrk.tile([41, 128], F16, tag="g3oh1", name="g3oh1")
        nc.vector.tensor_scalar(out=oh1[:], in0=psb[:41, :], scalar1=IOTA3[:41, 1:2],
                                scalar2=None, op0=mybir.AluOpType.is_equal)
        g3p = ps_g.tile([128, 256], F32, tag="pg", name="pg")
        nc.tensor.matmul(g3p[:], lhsT=oh0[:], rhs=K3c[0], start=True, stop=False)
        nc.tensor.matmul(g3p[:], lhsT=oh1[:], rhs=K3c[1], start=False, stop=True)
        nc.scalar.copy(G(3, t, 256 * p, 256 * p + 256), g3p[:])

    # head_w softmax over the 9 mixture logits (per channel chunk)
    HWH = []
    BASE = []
    BV2HW = []
    for ch in range(2):
        mx = wrk.tile([128, 1], F32, tag="mx", name="mx")
        nc.vector.reduce_max(mx[:], WMIX[ch], axis=mybir.AxisListType.X)
        nmx = wrk.tile([128, 1], F32, tag="nmx", name="nmx")
        nc.vector.tensor_scalar_mul(nmx[:], mx[:], -1.0)
        ex = wrk.tile([128, 9], F32, tag="ex", name="ex")
        sm = wrk.tile([128, 1], F32, tag="sm", name="sm")
        nc.scalar.activation(ex[:], WMIX[ch], mybir.ActivationFunctionType.Exp,
                             bias=nmx[:], accum_out=sm[:])
        rs = wrk.tile([128, 1], F32, tag="rs", name="rs")
        nc.vector.reciprocal(rs[:], sm[:])
        hw = csttile([128, 2], tag=f"hw{ch}")
        nc.vector.tensor_scalar_mul(hw[:], ex[:, 0:2], rs[:])
        HWH.append(hw[:, 0:1])
        base = csttile([128, 1], tag=f"base{ch}")
        nc.vector.tensor_tensor(out=base[:], in0=hw[:, 1:2], in1=FLG,
                                op=mybir.AluOpType.mult)
        BASE.append(base)
        b2h = csttile([128, 1], tag=f"b2h{ch}")
        nc.vector.tensor_tensor(out=b2h[:], in0=BV2[ch], in1=hw[:, 0:1],
                                op=mybir.AluOpType.mult)
        BV2HW.append(b2h)

    # ki_T = simil_add applied to add_keys (c2 x Lx), fp16 for fast Tadd
    KiT = [csttile([128, LX], F16, tag=f"kit{m}") for m in range(2)]
    for m in range(2):
        pps = ps_c.tile([128, LX], F32, tag="pc", name="pc")
        for dch in range(2):
            nc.tensor.matmul(pps[:], lhsT=SimT[4][dch][:, 128 * m:128 * m + 128],
                             rhs=AddkT[dch], start=(dch == 0), stop=(dch == 1))
        nc.scalar.copy(KiT[m][:], pps[:])

    # v2 = add_keys @ W_val[2h+1].T   (Lx x C)
    V2 = [csttile([128, C], F16, tag=f"v2{m}") for m in range(2)]
    for m in range(2):
        pps = ps_c.tile([128, C], F32, tag="pc", name="pc")
        for dch in range(2):
            nc.tensor.matmul(pps[:], lhsT=AddkT[dch][:, 128 * m:128 * m + 128],
                             rhs=Wv2T[dch], start=(dch == 0), stop=(dch == 1))
        nc.scalar.copy(V2[m][:], pps[:])

    # add_keys scores, exp(x-16), and early unnormalized transposes
    WADD = [csttile([128, LX], F16, tag=f"wadd{t}") for t in range(NT)]
    ZADD = [csttile([128, 1], tag=f"zadd{t}") for t in range(NT)]
    ZL = [csttile([128, 1], tag=f"zl{t}") for t in range(NT)]
    V = [csttile([128, C], F16, tag=f"v{t}") for t in range(NT)]
    WAT = [cst.tile([128, LQ], F16, tag=f"wat{ch}", name=f"wat{ch}")
           for ch in range(2)]
    for t in range(NT):
        qsl = slice(128 * t, 128 * t + 128)
        pta = ps_c.tile([128, LX], F32, tag="pc", name="pc")
        for ch in range(2):
            nc.tensor.matmul(pta[:], lhsT=QT16[ch][:, qsl], rhs=KiT[ch][:],
                             start=(ch == 0), stop=(ch == 1))
        nc.scalar.activation(WADD[t][:], pta[:], mybir.ActivationFunctionType.Exp,
                             bias=NEG16[:], accum_out=ZADD[t][:])
        for ch in range(2):
            fsl = slice(128 * ch, 128 * ch + 128)
            tp3 = ps_g.tile([128, 128], F16, tag="pg", name="pg")
            nc.tensor.transpose(out=tp3[:], in_=WADD[t][:, fsl], identity=IDENT16[:])
            nc.scalar.copy(WAT[ch][:, qsl], tp3[:])

    # ------- phase 3: per-level scores + DVE value accumulation ------------
    # Level order: 3 first (its G comes from the dense one-hot path and is
    # ready before any gathers land), then the gathered levels; the last
    # level also performs the per-tile normalization/transpose tail.
    S1T = cst.tile([1, LQ], F32R, tag="s1t", name="s1t")
    ZR = cst.tile([1, LQ], F32, tag="zr", name="zr")
    VT = [cst.tile([128, LQ], F16, tag=f"vt{ch}", name=f"vt{ch}")
          for ch in range(2)]
    SALL2 = [cst.tile([4, LQ], F32, tag=f"sall{k}", name=f"sall{k}")
             for k in range(2)]
    LVL_ORDER = (3, 0, 1, 2) if DENSE_L3 else (0, 1, 2, 3)
    for li, lvl in enumerate(LVL_ORDER):
        first, lastl = (li == 0), (li == L - 1)
        SALL = SALL2[li % 2]
        if lvl == 3 and DENSE_L3:
            for t in range(NT):
                for p in range(P):
                    dense_g3(t, p)
        STB = [[stsb.tile([128, 512], F32R, tag=f"stb{b8}_{dch}",
                          name=f"stb{b8}_{dch}") for dch in range(2)]
               for b8 in range(2)]
        for b8 in range(2):
            for dch in range(2):
                sps = ps_st.tile([128, 512], F32, tag="pst", name="pst")
                for qb in range(8):
                    ql = 8 * b8 + qb
                    b, pp = ql % 4, ql // 4
                    for c2 in range(2):
                        nc.tensor.matmul(
                            sps[:, 64 * qb:64 * qb + 64],
                            lhsT=G(lvl, 2 * b + c2,
                                   256 * pp + 128 * dch, 256 * pp + 128 * dch + 128),
                            rhs=QT16v[c2][:, :, ql],
                            start=(c2 == 0), stop=(c2 == 1))
                nc.scalar.copy(STB[b8][dch][:], sps[:])
        for b8 in range(2):
            scp = ps_o.tile([4, 512], F32, tag="po", name="po")
            for ich in range(2):
                cps = ps_c.tile([128, 512], F32, tag="pc", name="pc")
                isl = slice(128 * ich, 128 * ich + 128)
                for dch in range(2):
                    nc.tensor.matmul(cps[:], lhsT=SimT[lvl][dch][:, isl],
                                     rhs=STB[b8][dch][:],
                                     start=(dch == 0), stop=(dch == 1))
                mskb = wrk.tile([128, 512], F32R, tag="mskb", name="mskb")
                nc.vector.tensor_tensor(out=mskb[:], in0=cps[:], in1=DMASK,
                                        op=mybir.AluOpType.mult)
                nc.tensor.matmul(scp[:], lhsT=OH48[:, 4 * ich:4 * ich + 4],
                                 rhs=mskb[:], start=(ich == 0), stop=(ich == 1))
            sview = SALL[:].rearrange("p (t s) -> p s t", s=16)
            nc.scalar.copy(sview[:, 8 * b8:8 * b8 + 8, :], scp[:])

        for t in range(NT):
            tps = ps_tp.tile([128, 128], F32, tag="ptp", name="ptp")
            nc.tensor.transpose(out=tps[:, :4], in_=SALL[:, 128 * t:128 * t + 128],
                                identity=IDENT[:4, :4])
            wq = wrk.tile([128, 4], F32, tag="wq", name="wq")
            zp = wrk.tile([128, 1], F32, tag="zp", name="zp")
            nc.scalar.activation(wq[:], tps[:, :4], mybir.ActivationFunctionType.Exp,
                                 bias=NEG16[:], accum_out=zp[:])
            if first:
                nc.vector.tensor_copy(ZL[t][:], zp[:])
                nc.vector.tensor_scalar_mul(V[t][:], G(lvl, t, 0, 256), wq[:, 0:1])
            else:
                nc.vector.tensor_tensor(out=ZL[t][:], in0=ZL[t][:], in1=zp[:],
                                        op=mybir.AluOpType.add)
                nc.vector.scalar_tensor_tensor(
                    out=V[t][:], in0=G(lvl, t, 0, 256), scalar=wq[:, 0:1],
                    in1=V[t][:], op0=mybir.AluOpType.mult, op1=mybir.AluOpType.add)
            for p in range(1, P):
                nc.vector.scalar_tensor_tensor(
                    out=V[t][:], in0=G(lvl, t, 256 * p, 256 * p + 256),
                    scalar=wq[:, p:p + 1], in1=V[t][:],
                    op0=mybir.AluOpType.mult, op1=mybir.AluOpType.add)
            if lastl:
                # per-tile normalization/transpose tail, overlapped with the
                # remaining tiles' score/value work
                qsl = slice(128 * t, 128 * t + 128)
                zt = wrk.tile([128, 1], F32, tag="zt", name="zt")
                nc.vector.tensor_tensor(out=zt[:], in0=ZL[t][:], in1=ZADD[t][:],
                                        op=mybir.AluOpType.add)
                rz = wrk.tile([128, 1], F32, tag="rz", name="rz")
                nc.vector.reciprocal(rz[:], zt[:])
                tpsz = ps_tp.tile([128, 128], F32, tag="ptp", name="ptp")
                nc.tensor.transpose(out=tpsz[:1, :], in_=ZL[t][:], identity=IDENT[:])
                nc.scalar.copy(S1T[:, qsl], tpsz[:1, :])
                tpz = ps_tp.tile([128, 128], F32, tag="ptp", name="ptp")
                nc.tensor.transpose(out=tpz[:1, :], in_=rz[:], identity=IDENT[:])
                nc.scalar.copy(ZR[:, qsl], tpz[:1, :])
                for ch in range(2):
                    fsl = slice(128 * ch, 128 * ch + 128)
                    tp2 = ps_g.tile([128, 128], F16, tag="pg", name="pg")
                    nc.tensor.transpose(out=tp2[:], in_=V[t][:, fsl],
                                        identity=IDENT16[:])
                    nc.scalar.copy(VT[ch][:, qsl], tp2[:])

    # ------- phase 4 (tail): output matmuls ---------------------------------
    RES = [cst.tile([128, LQ], F16, tag=f"res{m}", name=f"res{m}") for m in range(2)]
    for n in range(2):
        nsl = slice(512 * n, 512 * n + 512)
        for m in range(2):
            msl = slice(128 * m, 128 * m + 128)
            rzb = ps_st.tile([128, 512], F32, tag="pst", name="pst")
            nc.tensor.matmul(rzb[:], lhsT=ONE1[:], rhs=ZR[:, nsl],
                             start=True, stop=True)
            ops = ps_o.tile([128, 512], F32, tag="po", name="po")
            nc.tensor.matmul(ops[:], lhsT=WvT16[0][:, msl], rhs=VT[0][:, nsl],
                             start=True, stop=False)
            nc.tensor.matmul(ops[:], lhsT=WvT16[1][:, msl], rhs=VT[1][:, nsl],
                             start=False, stop=False)
            nc.tensor.matmul(ops[:], lhsT=BVD[:, msl], rhs=S1T[:, nsl],
                             start=False, stop=False)
            nc.tensor.matmul(ops[:], lhsT=V2[0][:, msl], rhs=WAT[0][:, nsl],
                             start=False, stop=False)
            nc.tensor.matmul(ops[:], lhsT=V2[1][:, msl], rhs=WAT[1][:, nsl],
                             start=False, stop=True)
            sc1 = wrk.tile([128, 512], F32, tag="sc1", name="sc1")
            nc.scalar.activation(sc1[:], ops[:],
                                 mybir.ActivationFunctionType.Copy, scale=HWH[m])
            nc.vector.tensor_tensor(out=sc1[:], in0=sc1[:], in1=rzb[:],
                                    op=mybir.AluOpType.mult)
            bt = wrk.tile([128, 512], F32, tag="bt", name="bt")
            nc.scalar.activation(bt[:], QTF[:, 1024 * m + 512 * n:1024 * m + 512 * n + 512],
                                 mybir.ActivationFunctionType.Copy, scale=BASE[m][:])
            nc.vector.tensor_tensor(out=sc1[:], in0=sc1[:], in1=bt[:],
                                    op=mybir.AluOpType.add)
            nc.vector.tensor_scalar_add(RES[m][:, nsl], sc1[:], BV2HW[m][:])
            nc.sync.dma_start(d_out[msl, nsl], RES[m][:, nsl])


def _host_prepare(inputs):
    """Build per-core input maps from the full problem inputs."""
    q = np.asarray(inputs["query"], np.float32)[0]            # (1024, 256)
    rp = np.asarray(inputs["reference_points"], np.float32)[0]
    flat = np.ascontiguousarray(np.asarray(inputs["input_flatten"], np.float32)[0])
    iss = np.asarray(inputs["input_spatial_shapes"], np.int32)
    addk = np.asarray(inputs["add_keys"], np.float32)[0]
    lvst = np.asarray(inputs["input_level_start_index"], np.int32)
    W_off = np.asarray(inputs["W_off"], np.float32)
    b_off = np.asarray(inputs["b_off"], np.float32)
    W_attn = np.asarray(inputs["W_attn"], np.float32)
    W_val = np.asarray(inputs["W_val"], np.float32)
    b_val = np.asarray(inputs["b_val"], np.float32)
    W_mix = np.asarray(inputs["W_mix"], np.float32)

    iss_f = iss.astype(np.float32)
    wh = iss_f[:, ::-1]                                       # (W_l, H_l)
    inv_wh32 = np.repeat((1.0 / wh)[:, None, :], P, 1).reshape(32)
    iss32 = np.repeat(iss_f[:, None, :], P, 1).reshape(32)
    hl16 = np.repeat(iss[:, 0][:, None], P, 1).reshape(16)
    lv16 = np.repeat(lvst[:, None], P, 1).reshape(16)
    rp_rep = np.repeat(rp[:, :, None, :], P, 2).reshape(LQ, 32)
    wh32 = np.repeat(wh[:, None, :], P, 1).reshape(32)

    qT = np.ascontiguousarray(q.T)                            # (256, 1024)
    qt32 = np.concatenate([qT[:128], qT[128:]], axis=1)       # (128, 2048)
    qt16 = qt32.astype(np.float16)

    # ---- blob A (f32): idx-chain constants + W_off ----
    ca = np.zeros((128, NA), np.float32)
    ca[:, A_INVWH:A_INVWH + 256] = np.tile(inv_wh32, (128, 8))
    ca[:, A_ISSF:A_ISSF + 256] = np.tile(iss32, (128, 8))
    ca[:, A_WH32:A_WH32 + 256] = np.tile(wh32, (128, 8))
    ca[:, A_RP:A_RP + 256] = rp_rep.reshape(8, 128, 32).transpose(1, 0, 2).reshape(128, 256)
    iota3 = np.zeros((128, 2), np.float32)
    iota3[:, 0] = np.arange(128)
    iota3[:, 1] = 128 + np.arange(128)
    ca[:, A_IOTA3:A_IOTA3 + 2] = iota3

    # ---- blob B (f32): dmask/oh48/wmix/flag/bv2 ----
    cb = np.zeros((128, NB), np.float32)
    dm = np.zeros((128, 512), np.float32)
    for rr in range(128):
        dm[rr, rr % 64::64] = 1.0
    cb[:, B_DMASK:B_DMASK + 512] = dm
    oh = np.zeros((128, 8), np.float32)
    for rr in range(128):
        oh[rr, rr // 64] = 1.0          # ich 0: i//64 = p
        oh[rr, 4 + 2 + rr // 64] = 1.0  # ich 1: p = 2 + i'//64

    # ---- blob I (i32) ----
    ci = np.zeros((128, 256), np.int32)
    ci[:, 0:128] = np.tile(hl16, (128, 8))
    ci[:, 128:256] = np.tile(lv16, (128, 8))

    addkT = np.ascontiguousarray(addk.T)                      # (256, 256)

    in_maps = []
    for h in range(H):
        boff = b_off[32 * h:32 * h + 32]
        order = [h, 8] + [k for k in range(9) if k not in (h, 8)]
        wmix_r = np.ascontiguousarray(W_mix[:, order])        # (256, 9)

        cah = ca.copy()
        cah[:, A_BOFF:A_BOFF + 256] = np.tile(boff, (128, 8))
        woffT = np.ascontiguousarray(W_off[32 * h:32 * h + 32].T)  # (256, 32)
        cah[:, A_WOFF0:A_WOFF0 + 32] = woffT[:128]
        cah[:, A_WOFF1:A_WOFF1 + 32] = woffT[128:]

        cbh = cb.copy()
        cbh[:, B_WMIX:B_WMIX + 9] = wmix_r[:128]
        cbh[:, B_WMIX1:B_WMIX1 + 9] = wmix_r[128:]
        cbh[:, B_FLAG] = 1.0 if h == 0 else 0.0
        bv2 = b_val[2 * h + 1]                                # (256,)
        cbh[:, B_BV20] = bv2[:128]
        cbh[:, B_BV21] = bv2[128:]

        wvT = np.ascontiguousarray(W_val[2 * h].T).astype(np.float16)  # (256,256)
        cs = np.zeros((128, NS), np.float16)
        cs[:, S_QT16:S_QT16 + 2048] = qt16
        cs[:, S_WVT:S_WVT + 256] = wvT[:128]
        cs[:, S_WVT + 256:S_WVT + 512] = wvT[128:]
        k3 = flat.astype(np.float16)[LV3_START:LV3_START + LV3_ROWS]
        cs[:, S_K3:S_K3 + 256] = k3[:128]
        cs[:41, S_K3 + 256:S_K3 + 512] = k3[128:]

        wattnT = np.transpose(W_attn[4 * h:4 * h + 5], (0, 2, 1))  # (5,256,256)
        cr = np.zeros((128, NR), np.float32)
        for i in range(5):
            cr[:, R_SIM + 512 * i:R_SIM + 512 * i + 256] = wattnT[i, :128]
            cr[:, R_SIM + 512 * i + 256:R_SIM + 512 * i + 512] = wattnT[i, 128:]
        cr[:, R_ADDK:R_ADDK + 256] = addkT[:128]
        cr[:, R_ADDK + 256:R_ADDK + 512] = addkT[128:]
        wv2T = np.ascontiguousarray(W_val[2 * h + 1].T)
        cr[:, R_WV2:R_WV2 + 256] = wv2T[:128]
        cr[:, R_WV2 + 256:R_WV2 + 512] = wv2T[128:]
        cr[:, R_OH:R_OH + 8] = oh

        m = {
            "qt32": qt32,
            "cst32a": cah,
            "cst32b": cbh,
            "csti": ci,
            "cst16": cs,
            "cstr": cr,
            "bvd": (b_val[2 * h] - b_val[2 * h + 1]).reshape(1, C).astype(np.float32),
            "flatten16": flat.astype(np.float16),
        }
        in_maps.append(m)
    return in_maps


_CACHE = {}


def _get_nc():
    if "nc" not in _CACHE:
        nc = bacc.Bacc("TRN2", target_bir_lowering=False, debug=False)
        with tile.TileContext(nc) as tc:
            with ExitStack() as ctx:
                build_kernel(nc, tc, ctx)
        nc.compile()
        _CACHE["nc"] = nc
    return _CACHE["nc"]


def kernel(**inputs):
    nc = _get_nc()
    in_maps = _host_prepare(inputs)
    res = run_bass_kernel_spmd(nc, in_maps, core_ids=list(range(8)))
    total = np.zeros((C, LQ), np.float32)
    for h in range(H):
        total = total + res.results[h]["outT"].astype(np.float32)
    return np.ascontiguousarray(total.T)[None].astype(np.float32)


if __name__ == "__main__":
    import reference as R
    import jax.numpy as jnp

    inputs = {k: np.asarray(v) for k, v in R.setup_inputs().items()}
    out = kernel(**inputs)
    exp = np.asarray(R.reference(**{k: jnp.asarray(v) for k, v in inputs.items()}))
    err = np.abs(out - exp)
    scale = np.abs(exp).max()
    print("max abs err:", err.max(), "scale:", scale, "rel:", err.max() / scale)
